# revision 1
# baseline (speedup 1.0000x reference)
"""GroupedExperts (MoE bmm path) forward on 8 Trainium2 NeuronCores.

Reference (per expert e):
    h   = silu(x[e] @ w1[e]) * (x[e] @ w3[e])
    out = h @ w2[e]
with E=8, T=4096, D=2048, H=1024, fp32 inputs.

Sharding: expert-parallel — core e owns expert e (no cross-core traffic).

Device kernel design (per core):
  Host stages inputs as bf16 with x pre-transposed to xT [D, T] so every
  matmul consumes its natural layout (no on-device transposes):
    m1/m2: aT/bT[hm, tblk] = sum_dk w1/w3[dk, hm].T @ xT[dk, tblk]
           (lhsT = weight tile [128(D) x 128(H)], rhs = xT tile [128(D) x 512(T)])
    hT    = silu(aT) * bT                     (ACT + DVE, bf16 result)
    m3:    out[tm, dn] = sum_hk hT[hk, tm].T @ w2[hk, dn]
           (lhsT = hT tile [128(H) x 128(T)], rhs = w2 tile [128(H) x 512(D)])
  PSUM accumulates in fp32; out is written fp32 in natural [T, D] layout.
  Weights stay SBUF-resident (~96KB/partition); xT / hT are double-buffered
  per 512-token block so DMA and PE overlap.
"""

import numpy as np
import ml_dtypes

import concourse.bass as bass
import concourse.mybir as mybir
import concourse.tile as tile
from concourse import bacc
from concourse.bass_utils import run_bass_kernel_spmd

E, T, D, H = 8, 4096, 2048, 1024
NCORES = 8
P = 128               # partition dim
TBLK = 512            # token block = moving free dim for m1/m2
NTBLK = T // TBLK     # 8
NDK = D // P          # 16 contraction tiles over D
NHM = H // P          # 8 tiles over H
DBLK = 512            # D chunk = moving free dim for m3
NDN = D // DBLK       # 4
NTSUB = TBLK // P     # 4

BF16 = mybir.dt.bfloat16
F32 = mybir.dt.float32

_CACHE: dict = {}


def _build_module():
    if "nc" in _CACHE:
        return _CACHE["nc"]

    nc = bacc.Bacc(
        "TRN2",
        target_bir_lowering=False,
        debug=False,
        enable_asserts=False,
        num_devices=NCORES,
    )

    xt_d = nc.dram_tensor("xt", [D, T], BF16, kind="ExternalInput").ap()
    w1_d = nc.dram_tensor("w1", [D, H], BF16, kind="ExternalInput").ap()
    w3_d = nc.dram_tensor("w3", [D, H], BF16, kind="ExternalInput").ap()
    w2_d = nc.dram_tensor("w2", [H, D], BF16, kind="ExternalInput").ap()
    out_d = nc.dram_tensor("out", [T, D], F32, kind="ExternalOutput").ap()

    with tile.TileContext(nc) as tc:
        with (
            tc.tile_pool(name="wpool", bufs=1) as wpool,
            tc.tile_pool(name="xpool", bufs=2) as xpool,
            tc.tile_pool(name="hpool", bufs=2) as hpool,
            tc.tile_pool(name="spool", bufs=4) as spool,
            tc.tile_pool(name="opool", bufs=4) as opool,
            tc.tile_pool(name="psab", bufs=2, space="PSUM") as psab,
            tc.tile_pool(name="pso", bufs=2, space="PSUM") as pso,
        ):
            # Resident weights: distinct tags -> one persistent slot each.
            w1_t = []
            w3_t = []
            for k in range(NDK):
                t1 = wpool.tile([P, H], BF16, tag=f"w1_{k}")
                t3 = wpool.tile([P, H], BF16, tag=f"w3_{k}")
                nc.sync.dma_start(t1[:], w1_d[k * P:(k + 1) * P, :])
                nc.sync.dma_start(t3[:], w3_d[k * P:(k + 1) * P, :])
                w1_t.append(t1)
                w3_t.append(t3)
            w2_t = []
            for k in range(NHM):
                t2 = wpool.tile([P, D], BF16, tag=f"w2_{k}")
                nc.sync.dma_start(t2[:], w2_d[k * P:(k + 1) * P, :])
                w2_t.append(t2)

            for i in range(NTBLK):
                ts = i * TBLK
                xt_t = []
                for k in range(NDK):
                    xt = xpool.tile([P, TBLK], BF16, tag=f"x_{k}")
                    nc.sync.dma_start(
                        xt[:], xt_d[k * P:(k + 1) * P, ts:ts + TBLK]
                    )
                    xt_t.append(xt)

                hts = []
                for hm in range(NHM):
                    hs = hm * P
                    pa = psab.tile([P, TBLK], F32, tag="pa")
                    pb = psab.tile([P, TBLK], F32, tag="pb")
                    for k in range(NDK):
                        nc.tensor.matmul(
                            pa[:], w1_t[k][:, hs:hs + P], xt_t[k][:],
                            start=(k == 0), stop=(k == NDK - 1),
                        )
                    for k in range(NDK):
                        nc.tensor.matmul(
                            pb[:], w3_t[k][:, hs:hs + P], xt_t[k][:],
                            start=(k == 0), stop=(k == NDK - 1),
                        )
                    sil = spool.tile([P, TBLK], BF16, tag="sil")
                    nc.scalar.activation(
                        sil[:], pa[:], mybir.ActivationFunctionType.Silu
                    )
                    ht = hpool.tile([P, TBLK], BF16, tag=f"h_{hm}")
                    nc.vector.tensor_mul(ht[:], sil[:], pb[:])
                    hts.append(ht)

                for tm in range(NTSUB):
                    tsub = ts + tm * P
                    for dn in range(NDN):
                        dsl = dn * DBLK
                        po = pso.tile([P, DBLK], F32, tag="po")
                        for hk in range(NHM):
                            nc.tensor.matmul(
                                po[:],
                                hts[hk][:, tm * P:(tm + 1) * P],
                                w2_t[hk][:, dsl:dsl + DBLK],
                                start=(hk == 0), stop=(hk == NHM - 1),
                            )
                        ot = opool.tile([P, DBLK], F32, tag="o")
                        nc.vector.tensor_copy(ot[:], po[:])
                        nc.sync.dma_start(
                            out_d[tsub:tsub + P, dsl:dsl + DBLK], ot[:]
                        )

    nc.compile()
    _CACHE["nc"] = nc
    return nc


def _stage_inputs(x, w1, w2, w3):
    """Per-expert bf16 staging; x pre-transposed to [D, T]."""
    bf = ml_dtypes.bfloat16
    in_maps = []
    for e in range(E):
        in_maps.append({
            "xt": np.ascontiguousarray(x[e].astype(bf).T),
            "w1": np.ascontiguousarray(w1[e].astype(bf)),
            "w3": np.ascontiguousarray(w3[e].astype(bf)),
            "w2": np.ascontiguousarray(w2[e].astype(bf)),
        })
    return in_maps


def kernel(x, w1, w2, w3):
    assert x.shape == (E, T, D) and w1.shape == (E, D, H)
    assert w2.shape == (E, H, D) and w3.shape == (E, D, H)
    nc = _build_module()
    in_maps = _stage_inputs(x, w1, w2, w3)
    res = run_bass_kernel_spmd(nc, in_maps, core_ids=list(range(NCORES)))
    out = np.stack([res.results[e]["out"] for e in range(E)], axis=0)
    return out.astype(np.float32)



# revision 2
# speedup vs baseline: 7.8136x; 7.8136x over previous
"""GroupedExperts (MoE bmm path) forward on 8 Trainium2 NeuronCores.

Reference (per expert e):
    h   = silu(x[e] @ w1[e]) * (x[e] @ w3[e])
    out = h @ w2[e]
with E=8, T=4096, D=2048, H=1024, fp32 inputs.

Sharding: expert-parallel - core e owns expert e (no cross-core traffic).

v2 changes vs v1:
  - out written bf16 (host upcasts): halves output HBM+staging bytes;
    adds ~2e-4 rel err, well within budget.
  - software-pipelined m3: block i's h@w2 matmuls are issued after block
    i+1's m1/m2 matmuls, so the PE never waits on ACT/DVE producing h.
  - DMA issue order w1 -> x(0) -> w3 -> w2 so the PE can start m1 as
    early as possible.
"""

import numpy as np
import ml_dtypes

import concourse.bass as bass
import concourse.mybir as mybir
import concourse.tile as tile
from concourse import bacc
from concourse.bass_utils import run_bass_kernel_spmd

E, T, D, H = 8, 4096, 2048, 1024
NCORES = 8
P = 128               # partition dim
TBLK = 512            # token block = moving free dim for m1/m2
NTBLK = T // TBLK     # 8
NDK = D // P          # 16 contraction tiles over D
NHM = H // P          # 8 tiles over H
DBLK = 512            # D chunk = moving free dim for m3
NDN = D // DBLK       # 4
NTSUB = TBLK // P     # 4

BF16 = mybir.dt.bfloat16
F32 = mybir.dt.float32

_CACHE: dict = {}


def _build_module():
    if "nc" in _CACHE:
        return _CACHE["nc"]

    nc = bacc.Bacc(
        "TRN2",
        target_bir_lowering=False,
        debug=False,
        enable_asserts=False,
        num_devices=NCORES,
    )

    xt_d = nc.dram_tensor("xt", [D, T], BF16, kind="ExternalInput").ap()
    w1_d = nc.dram_tensor("w1", [D, H], BF16, kind="ExternalInput").ap()
    w3_d = nc.dram_tensor("w3", [D, H], BF16, kind="ExternalInput").ap()
    w2_d = nc.dram_tensor("w2", [H, D], BF16, kind="ExternalInput").ap()
    out_d = nc.dram_tensor("out", [T, D], BF16, kind="ExternalOutput").ap()

    with tile.TileContext(nc) as tc:
        with (
            tc.tile_pool(name="wpool", bufs=1) as wpool,
            tc.tile_pool(name="xpool", bufs=2) as xpool,
            tc.tile_pool(name="hpool", bufs=2) as hpool,
            tc.tile_pool(name="spool", bufs=2) as spool,
            tc.tile_pool(name="opool", bufs=4) as opool,
            tc.tile_pool(name="psab", bufs=2, space="PSUM") as psab,
            tc.tile_pool(name="pso", bufs=2, space="PSUM") as pso,
        ):
            # Resident weights. DMA issue order: (w1_k, x0_k) pairs so the
            # first m1 accumulation group can start after ~one pair lands,
            # then w3, w2.
            w1_t = []
            x0_t = []
            for k in range(NDK):
                t1 = wpool.tile([P, H], BF16, tag=f"w1_{k}")
                nc.sync.dma_start(t1[:], w1_d[k * P:(k + 1) * P, :])
                w1_t.append(t1)
                xt = xpool.tile([P, TBLK], BF16, tag=f"x_{k}")
                nc.sync.dma_start(xt[:], xt_d[k * P:(k + 1) * P, 0:TBLK])
                x0_t.append(xt)

            def load_w3_w2():
                w3_t, w2_t = [], []
                for k in range(NDK):
                    t3 = wpool.tile([P, H], BF16, tag=f"w3_{k}")
                    nc.sync.dma_start(t3[:], w3_d[k * P:(k + 1) * P, :])
                    w3_t.append(t3)
                for k in range(NHM):
                    t2 = wpool.tile([P, D], BF16, tag=f"w2_{k}")
                    nc.sync.dma_start(t2[:], w2_d[k * P:(k + 1) * P, :])
                    w2_t.append(t2)
                return w3_t, w2_t

            w3_t = w2_t = None

            def load_x(i):
                ts = i * TBLK
                xt_t = []
                for k in range(NDK):
                    xt = xpool.tile([P, TBLK], BF16, tag=f"x_{k}")
                    nc.sync.dma_start(
                        xt[:], xt_d[k * P:(k + 1) * P, ts:ts + TBLK]
                    )
                    xt_t.append(xt)
                return xt_t

            def emit_m3(i, hts):
                """h(i) @ w2 -> out rows [i*TBLK, (i+1)*TBLK)."""
                ts = i * TBLK
                for tm in range(NTSUB):
                    tsub = ts + tm * P
                    for dn in range(NDN):
                        dsl = dn * DBLK
                        po = pso.tile([P, DBLK], F32, tag="po")
                        for hk in range(NHM):
                            nc.tensor.matmul(
                                po[:],
                                hts[hk][:, tm * P:(tm + 1) * P],
                                w2_t[hk][:, dsl:dsl + DBLK],
                                start=(hk == 0), stop=(hk == NHM - 1),
                            )
                        ot = opool.tile([P, DBLK], BF16, tag="o")
                        nc.vector.tensor_copy(ot[:], po[:])
                        nc.sync.dma_start(
                            out_d[tsub:tsub + P, dsl:dsl + DBLK], ot[:]
                        )

            prev_hts = None
            for i in range(NTBLK):
                if i == 0:
                    xt_t = x0_t
                    # issued after x(0) so the first m1 group isn't
                    # queued behind 8MB of w3/w2 traffic.
                    w3_t, w2_t = load_w3_w2()
                else:
                    xt_t = load_x(i)

                # Pass 1: all m1 groups (need only w1 + x), silu on ACT.
                sils = []
                for hm in range(NHM):
                    hs = hm * P
                    pa = psab.tile([P, TBLK], F32, tag="pa")
                    for k in range(NDK):
                        nc.tensor.matmul(
                            pa[:], w1_t[k][:, hs:hs + P], xt_t[k][:],
                            start=(k == 0), stop=(k == NDK - 1),
                        )
                    sil = spool.tile([P, TBLK], BF16, tag=f"sil_{hm}")
                    nc.scalar.activation(
                        sil[:], pa[:], mybir.ActivationFunctionType.Silu
                    )
                    sils.append(sil)
                # Pass 2: all m2 groups (need w3), DVE mul -> h.
                hts = []
                for hm in range(NHM):
                    hs = hm * P
                    pb = psab.tile([P, TBLK], F32, tag="pb")
                    for k in range(NDK):
                        nc.tensor.matmul(
                            pb[:], w3_t[k][:, hs:hs + P], xt_t[k][:],
                            start=(k == 0), stop=(k == NDK - 1),
                        )
                    ht = hpool.tile([P, TBLK], BF16, tag=f"h_{hm}")
                    nc.vector.tensor_mul(ht[:], sils[hm][:], pb[:])
                    hts.append(ht)

                if prev_hts is not None:
                    emit_m3(i - 1, prev_hts)
                prev_hts = hts

            emit_m3(NTBLK - 1, prev_hts)

    nc.compile()
    _CACHE["nc"] = nc
    return nc


def _stage_inputs(x, w1, w2, w3):
    """Per-expert bf16 staging; x pre-transposed to [D, T]."""
    bf = ml_dtypes.bfloat16
    in_maps = []
    for e in range(E):
        in_maps.append({
            "xt": np.ascontiguousarray(x[e].astype(bf).T),
            "w1": np.ascontiguousarray(w1[e].astype(bf)),
            "w3": np.ascontiguousarray(w3[e].astype(bf)),
            "w2": np.ascontiguousarray(w2[e].astype(bf)),
        })
    return in_maps


def kernel(x, w1, w2, w3):
    assert x.shape == (E, T, D) and w1.shape == (E, D, H)
    assert w2.shape == (E, H, D) and w3.shape == (E, D, H)
    nc = _build_module()
    in_maps = _stage_inputs(x, w1, w2, w3)
    res = run_bass_kernel_spmd(nc, in_maps, core_ids=list(range(NCORES)))
    out = np.stack([res.results[e]["out"] for e in range(E)], axis=0)
    return out.astype(np.float32)


# revision 4
# speedup vs baseline: 7.8265x; 1.0017x over previous
"""GroupedExperts (MoE bmm path) forward on 8 Trainium2 NeuronCores.

Reference (per expert e):
    h   = silu(x[e] @ w1[e]) * (x[e] @ w3[e])
    out = h @ w2[e]
with E=8, T=4096, D=2048, H=1024, fp32 inputs.

Sharding: expert-parallel - core e owns expert e (no cross-core traffic).

Schedule (measured 693 us/core on HW, vs 655 us bf16 PE roofline):
  - out written bf16 (host upcasts): halves output HBM+staging bytes;
    adds ~2e-4 rel err, well within budget.
  - software-pipelined m3: block i's h@w2 matmuls are issued after block
    i+1's m1/m2 matmuls, so the PE never waits on ACT/DVE producing h.
  - startup DMAs interleaved as (w1_k, x0_k) pairs and block 0 runs all
    m1 groups before any m2 group, hiding the w3/w2 loads behind m1.
  - pa/pb PSUM pools triple-buffered (3+3+2 = 8 banks) to keep PSUM
    group-boundary WAR waits off the PE queue.
"""

import numpy as np
import ml_dtypes

import concourse.bass as bass
import concourse.mybir as mybir
import concourse.tile as tile
from concourse import bacc
from concourse.bass_utils import run_bass_kernel_spmd

E, T, D, H = 8, 4096, 2048, 1024
NCORES = 8
P = 128               # partition dim
TBLK = 512            # token block = moving free dim for m1/m2
NTBLK = T // TBLK     # 8
NDK = D // P          # 16 contraction tiles over D
NHM = H // P          # 8 tiles over H
DBLK = 512            # D chunk = moving free dim for m3
NDN = D // DBLK       # 4
NTSUB = TBLK // P     # 4

BF16 = mybir.dt.bfloat16
F32 = mybir.dt.float32

_CACHE: dict = {}


def _build_module():
    if "nc" in _CACHE:
        return _CACHE["nc"]

    nc = bacc.Bacc(
        "TRN2",
        target_bir_lowering=False,
        debug=False,
        enable_asserts=False,
        num_devices=NCORES,
    )

    xt_d = nc.dram_tensor("xt", [D, T], BF16, kind="ExternalInput").ap()
    w1_d = nc.dram_tensor("w1", [D, H], BF16, kind="ExternalInput").ap()
    w3_d = nc.dram_tensor("w3", [D, H], BF16, kind="ExternalInput").ap()
    w2_d = nc.dram_tensor("w2", [H, D], BF16, kind="ExternalInput").ap()
    out_d = nc.dram_tensor("out", [T, D], BF16, kind="ExternalOutput").ap()

    with tile.TileContext(nc) as tc:
        with (
            tc.tile_pool(name="wpool", bufs=1) as wpool,
            tc.tile_pool(name="xpool", bufs=2) as xpool,
            tc.tile_pool(name="hpool", bufs=2) as hpool,
            tc.tile_pool(name="spool", bufs=2) as spool,
            tc.tile_pool(name="opool", bufs=4) as opool,
            tc.tile_pool(name="psab", bufs=3, space="PSUM") as psab,
            tc.tile_pool(name="pso", bufs=2, space="PSUM") as pso,
        ):
            # Resident weights. DMA issue order: (w1_k, x0_k) pairs so the
            # first m1 accumulation group can start after ~one pair lands,
            # then w3, w2.
            w1_t = []
            x0_t = []
            for k in range(NDK):
                t1 = wpool.tile([P, H], BF16, tag=f"w1_{k}")
                nc.sync.dma_start(t1[:], w1_d[k * P:(k + 1) * P, :])
                w1_t.append(t1)
                xt = xpool.tile([P, TBLK], BF16, tag=f"x_{k}")
                nc.sync.dma_start(xt[:], xt_d[k * P:(k + 1) * P, 0:TBLK])
                x0_t.append(xt)

            def load_w3_w2():
                w3_t, w2_t = [], []
                for k in range(NDK):
                    t3 = wpool.tile([P, H], BF16, tag=f"w3_{k}")
                    nc.sync.dma_start(t3[:], w3_d[k * P:(k + 1) * P, :])
                    w3_t.append(t3)
                for k in range(NHM):
                    t2 = wpool.tile([P, D], BF16, tag=f"w2_{k}")
                    nc.sync.dma_start(t2[:], w2_d[k * P:(k + 1) * P, :])
                    w2_t.append(t2)
                return w3_t, w2_t

            w3_t = w2_t = None

            def load_x(i):
                ts = i * TBLK
                xt_t = []
                for k in range(NDK):
                    xt = xpool.tile([P, TBLK], BF16, tag=f"x_{k}")
                    nc.sync.dma_start(
                        xt[:], xt_d[k * P:(k + 1) * P, ts:ts + TBLK]
                    )
                    xt_t.append(xt)
                return xt_t

            def emit_m3(i, hts):
                """h(i) @ w2 -> out rows [i*TBLK, (i+1)*TBLK)."""
                ts = i * TBLK
                for tm in range(NTSUB):
                    tsub = ts + tm * P
                    for dn in range(NDN):
                        dsl = dn * DBLK
                        po = pso.tile([P, DBLK], F32, tag="po")
                        for hk in range(NHM):
                            nc.tensor.matmul(
                                po[:],
                                hts[hk][:, tm * P:(tm + 1) * P],
                                w2_t[hk][:, dsl:dsl + DBLK],
                                start=(hk == 0), stop=(hk == NHM - 1),
                            )
                        ot = opool.tile([P, DBLK], BF16, tag="o")
                        nc.vector.tensor_copy(ot[:], po[:])
                        nc.sync.dma_start(
                            out_d[tsub:tsub + P, dsl:dsl + DBLK], ot[:]
                        )

            prev_hts = None
            for i in range(NTBLK):
                if i == 0:
                    xt_t = x0_t
                    # issued after x(0) so the first m1 group isn't
                    # queued behind 8MB of w3/w2 traffic.
                    w3_t, w2_t = load_w3_w2()
                else:
                    xt_t = load_x(i)

                # Pass 1: all m1 groups (need only w1 + x), silu on ACT.
                sils = []
                for hm in range(NHM):
                    hs = hm * P
                    pa = psab.tile([P, TBLK], F32, tag="pa")
                    for k in range(NDK):
                        nc.tensor.matmul(
                            pa[:], w1_t[k][:, hs:hs + P], xt_t[k][:],
                            start=(k == 0), stop=(k == NDK - 1),
                        )
                    sil = spool.tile([P, TBLK], BF16, tag=f"sil_{hm}")
                    nc.scalar.activation(
                        sil[:], pa[:], mybir.ActivationFunctionType.Silu
                    )
                    sils.append(sil)
                # Pass 2: all m2 groups (need w3), DVE mul -> h.
                hts = []
                for hm in range(NHM):
                    hs = hm * P
                    pb = psab.tile([P, TBLK], F32, tag="pb")
                    for k in range(NDK):
                        nc.tensor.matmul(
                            pb[:], w3_t[k][:, hs:hs + P], xt_t[k][:],
                            start=(k == 0), stop=(k == NDK - 1),
                        )
                    ht = hpool.tile([P, TBLK], BF16, tag=f"h_{hm}")
                    nc.vector.tensor_mul(ht[:], sils[hm][:], pb[:])
                    hts.append(ht)

                if prev_hts is not None:
                    emit_m3(i - 1, prev_hts)
                prev_hts = hts

            emit_m3(NTBLK - 1, prev_hts)

    nc.compile()
    _CACHE["nc"] = nc
    return nc


def _stage_inputs(x, w1, w2, w3):
    """Per-expert bf16 staging; x pre-transposed to [D, T]."""
    bf = ml_dtypes.bfloat16
    in_maps = []
    for e in range(E):
        in_maps.append({
            "xt": np.ascontiguousarray(x[e].astype(bf).T),
            "w1": np.ascontiguousarray(w1[e].astype(bf)),
            "w3": np.ascontiguousarray(w3[e].astype(bf)),
            "w2": np.ascontiguousarray(w2[e].astype(bf)),
        })
    return in_maps


def kernel(x, w1, w2, w3):
    assert x.shape == (E, T, D) and w1.shape == (E, D, H)
    assert w2.shape == (E, H, D) and w3.shape == (E, D, H)
    nc = _build_module()
    in_maps = _stage_inputs(x, w1, w2, w3)
    res = run_bass_kernel_spmd(nc, in_maps, core_ids=list(range(NCORES)))
    out = np.stack([res.results[e]["out"] for e in range(E)], axis=0)
    return out.astype(np.float32)


# revision 11
# speedup vs baseline: 7.8530x; 1.0034x over previous
"""GroupedExperts (MoE bmm path) forward on 8 Trainium2 NeuronCores.

Reference (per expert e):
    h   = silu(x[e] @ w1[e]) * (x[e] @ w3[e])
    out = h @ w2[e]
with E=8, T=4096, D=2048, H=1024, fp32 inputs.

Sharding: expert-parallel - core e owns expert e (no cross-core traffic).

Schedule (measured ~691 us/core on HW, vs 655 us bf16 PE roofline):
  - out written bf16 (host upcasts): halves output HBM+staging bytes;
    adds ~2e-4 rel err, well within budget.
  - software-pipelined m3: block i's h@w2 matmuls are issued after block
    i+1's m1/m2 matmuls, so the PE never waits on ACT/DVE producing h.
  - startup DMAs interleaved as (w1_k, x0_k) pairs and block 0 runs all
    m1 groups before any m2 group, hiding the w3/w2 loads behind m1.
  - pa/pb PSUM pools triple-buffered (3+3+2 = 8 banks) to keep PSUM
    group-boundary WAR waits off the PE queue.
  - ~40 dummy warmup matmuls during the initial DMA wait release the
    HAM clock gate (1.2 -> 2.4 GHz) before real work arrives.
"""

import numpy as np
import ml_dtypes

import concourse.bass as bass
import concourse.mybir as mybir
import concourse.tile as tile
from concourse import bacc
from concourse.bass_utils import run_bass_kernel_spmd

E, T, D, H = 8, 4096, 2048, 1024
NCORES = 8
P = 128               # partition dim
TBLK = 512            # token block = moving free dim for m1/m2
NTBLK = T // TBLK     # 8
NDK = D // P          # 16 contraction tiles over D
NHM = H // P          # 8 tiles over H
DBLK = 512            # D chunk = moving free dim for m3
NDN = D // DBLK       # 4
NTSUB = TBLK // P     # 4

BF16 = mybir.dt.bfloat16
F32 = mybir.dt.float32

_CACHE: dict = {}


def _build_module():
    if "nc" in _CACHE:
        return _CACHE["nc"]

    nc = bacc.Bacc(
        "TRN2",
        target_bir_lowering=False,
        debug=False,
        enable_asserts=False,
        num_devices=NCORES,
    )

    xt_d = nc.dram_tensor("xt", [D, T], BF16, kind="ExternalInput").ap()
    w1_d = nc.dram_tensor("w1", [D, H], BF16, kind="ExternalInput").ap()
    w3_d = nc.dram_tensor("w3", [D, H], BF16, kind="ExternalInput").ap()
    w2_d = nc.dram_tensor("w2", [H, D], BF16, kind="ExternalInput").ap()
    out_d = nc.dram_tensor("out", [T, D], BF16, kind="ExternalOutput").ap()

    with tile.TileContext(nc) as tc:
        with (
            tc.tile_pool(name="wpool", bufs=1) as wpool,
            tc.tile_pool(name="xpool", bufs=2) as xpool,
            tc.tile_pool(name="hpool", bufs=2) as hpool,
            tc.tile_pool(name="spool", bufs=2) as spool,
            tc.tile_pool(name="opool", bufs=4) as opool,
            tc.tile_pool(name="psab", bufs=3, space="PSUM") as psab,
            tc.tile_pool(name="pso", bufs=2, space="PSUM") as pso,
        ):
            # HAM warmup: the PE clock sits at 1.2 GHz until ~3.4us of
            # sustained matmul activity. Burn dummy matmuls on a memset
            # tile while the first weight/x DMAs are in flight so the real
            # matmuls start at 2.4 GHz.
            wu = spool.tile([P, TBLK], BF16, tag="warm", bufs=1)
            nc.vector.memset(wu[:], 0.0)
            pwu = pso.tile([P, TBLK], F32, tag="po", name="pwu")
            for _ in range(40):
                nc.tensor.matmul(pwu[:], wu[:, 0:P], wu[:],
                                 start=True, stop=True)

            # Resident weights. DMA issue order: (w1_k, x0_k) pairs so the
            # first m1 accumulation group can start after ~one pair lands,
            # then w3, w2.
            w1_t = []
            x0_t = []
            for k in range(NDK):
                t1 = wpool.tile([P, H], BF16, tag=f"w1_{k}")
                nc.sync.dma_start(t1[:], w1_d[k * P:(k + 1) * P, :])
                w1_t.append(t1)
                xt = xpool.tile([P, TBLK], BF16, tag=f"x_{k}")
                nc.sync.dma_start(xt[:], xt_d[k * P:(k + 1) * P, 0:TBLK])
                x0_t.append(xt)

            def load_w3_w2():
                w3_t, w2_t = [], []
                for k in range(NDK):
                    t3 = wpool.tile([P, H], BF16, tag=f"w3_{k}")
                    nc.sync.dma_start(t3[:], w3_d[k * P:(k + 1) * P, :])
                    w3_t.append(t3)
                for k in range(NHM):
                    t2 = wpool.tile([P, D], BF16, tag=f"w2_{k}")
                    nc.sync.dma_start(t2[:], w2_d[k * P:(k + 1) * P, :])
                    w2_t.append(t2)
                return w3_t, w2_t

            w3_t = w2_t = None

            def load_x(i):
                ts = i * TBLK
                xt_t = []
                for k in range(NDK):
                    xt = xpool.tile([P, TBLK], BF16, tag=f"x_{k}")
                    nc.sync.dma_start(
                        xt[:], xt_d[k * P:(k + 1) * P, ts:ts + TBLK]
                    )
                    xt_t.append(xt)
                return xt_t

            def emit_m3(i, hts):
                """h(i) @ w2 -> out rows [i*TBLK, (i+1)*TBLK)."""
                ts = i * TBLK
                for tm in range(NTSUB):
                    tsub = ts + tm * P
                    for dn in range(NDN):
                        dsl = dn * DBLK
                        po = pso.tile([P, DBLK], F32, tag="po")
                        for hk in range(NHM):
                            nc.tensor.matmul(
                                po[:],
                                hts[hk][:, tm * P:(tm + 1) * P],
                                w2_t[hk][:, dsl:dsl + DBLK],
                                start=(hk == 0), stop=(hk == NHM - 1),
                            )
                        ot = opool.tile([P, DBLK], BF16, tag="o")
                        nc.vector.tensor_copy(ot[:], po[:])
                        nc.sync.dma_start(
                            out_d[tsub:tsub + P, dsl:dsl + DBLK], ot[:]
                        )

            prev_hts = None
            for i in range(NTBLK):
                if i == 0:
                    xt_t = x0_t
                    # issued after x(0) so the first m1 group isn't
                    # queued behind 8MB of w3/w2 traffic.
                    w3_t, w2_t = load_w3_w2()
                else:
                    xt_t = load_x(i)

                # Pass 1: all m1 groups (need only w1 + x), silu on ACT.
                sils = []
                for hm in range(NHM):
                    hs = hm * P
                    pa = psab.tile([P, TBLK], F32, tag="pa")
                    for k in range(NDK):
                        nc.tensor.matmul(
                            pa[:], w1_t[k][:, hs:hs + P], xt_t[k][:],
                            start=(k == 0), stop=(k == NDK - 1),
                        )
                    sil = spool.tile([P, TBLK], BF16, tag=f"sil_{hm}")
                    nc.scalar.activation(
                        sil[:], pa[:], mybir.ActivationFunctionType.Silu
                    )
                    sils.append(sil)
                # Pass 2: all m2 groups (need w3), DVE mul -> h.
                hts = []
                for hm in range(NHM):
                    hs = hm * P
                    pb = psab.tile([P, TBLK], F32, tag="pb")
                    for k in range(NDK):
                        nc.tensor.matmul(
                            pb[:], w3_t[k][:, hs:hs + P], xt_t[k][:],
                            start=(k == 0), stop=(k == NDK - 1),
                        )
                    ht = hpool.tile([P, TBLK], BF16, tag=f"h_{hm}")
                    nc.vector.tensor_mul(ht[:], sils[hm][:], pb[:])
                    hts.append(ht)

                if prev_hts is not None:
                    emit_m3(i - 1, prev_hts)
                prev_hts = hts

            emit_m3(NTBLK - 1, prev_hts)

    nc.compile()
    _CACHE["nc"] = nc
    return nc


def _stage_inputs(x, w1, w2, w3):
    """Per-expert bf16 staging; x pre-transposed to [D, T]."""
    bf = ml_dtypes.bfloat16
    in_maps = []
    for e in range(E):
        in_maps.append({
            "xt": np.ascontiguousarray(x[e].astype(bf).T),
            "w1": np.ascontiguousarray(w1[e].astype(bf)),
            "w3": np.ascontiguousarray(w3[e].astype(bf)),
            "w2": np.ascontiguousarray(w2[e].astype(bf)),
        })
    return in_maps


def kernel(x, w1, w2, w3):
    assert x.shape == (E, T, D) and w1.shape == (E, D, H)
    assert w2.shape == (E, H, D) and w3.shape == (E, D, H)
    nc = _build_module()
    in_maps = _stage_inputs(x, w1, w2, w3)
    res = run_bass_kernel_spmd(nc, in_maps, core_ids=list(range(NCORES)))
    out = np.stack([res.results[e]["out"] for e in range(E)], axis=0)
    return out.astype(np.float32)


# revision 15
# speedup vs baseline: 7.8609x; 1.0010x over previous
"""GroupedExperts (MoE bmm path) forward on 8 Trainium2 NeuronCores.

Reference (per expert e):
    h   = silu(x[e] @ w1[e]) * (x[e] @ w3[e])
    out = h @ w2[e]
with E=8, T=4096, D=2048, H=1024, fp32 inputs.

Sharding: expert-parallel - core e owns expert e (no cross-core traffic).

Schedule (measured ~691 us/core on HW, vs 655 us bf16 PE roofline):
  - out written bf16 (host upcasts): halves output HBM+staging bytes;
    adds ~2e-4 rel err, well within budget.
  - software-pipelined m3: block i's h@w2 matmuls are issued after block
    i+1's m1/m2 matmuls, so the PE never waits on ACT/DVE producing h.
  - startup DMAs interleaved as (w1_k, x0_k) pairs and block 0 runs all
    m1 groups before any m2 group, hiding the w3/w2 loads behind m1.
  - pa/pb PSUM pools triple-buffered (3+3+2 = 8 banks) to keep PSUM
    group-boundary WAR waits off the PE queue.
  - ~40 dummy warmup matmuls during the initial DMA wait release the
    HAM clock gate (1.2 -> 2.4 GHz) before real work arrives.
"""

import numpy as np
import ml_dtypes

import concourse.bass as bass
import concourse.mybir as mybir
import concourse.tile as tile
from concourse import bacc
from concourse.bass_utils import run_bass_kernel_spmd

E, T, D, H = 8, 4096, 2048, 1024
NCORES = 8
P = 128               # partition dim
TBLK = 512            # token block = moving free dim for m1/m2
NTBLK = T // TBLK     # 8
NDK = D // P          # 16 contraction tiles over D
NHM = H // P          # 8 tiles over H
DBLK = 512            # D chunk = moving free dim for m3
NDN = D // DBLK       # 4
NTSUB = TBLK // P     # 4

BF16 = mybir.dt.bfloat16
F32 = mybir.dt.float32

_CACHE: dict = {}


def _build_module():
    if "nc" in _CACHE:
        return _CACHE["nc"]

    nc = bacc.Bacc(
        "TRN2",
        target_bir_lowering=False,
        debug=False,
        enable_asserts=False,
        num_devices=NCORES,
    )

    xt_d = nc.dram_tensor("xt", [D, T], BF16, kind="ExternalInput").ap()
    w1_d = nc.dram_tensor("w1", [D, H], BF16, kind="ExternalInput").ap()
    w3_d = nc.dram_tensor("w3", [D, H], BF16, kind="ExternalInput").ap()
    w2_d = nc.dram_tensor("w2", [H, D], BF16, kind="ExternalInput").ap()
    out_d = nc.dram_tensor("out", [T, D], BF16, kind="ExternalOutput").ap()

    with tile.TileContext(nc) as tc:
        with (
            tc.tile_pool(name="wpool", bufs=1) as wpool,
            tc.tile_pool(name="xpool", bufs=2) as xpool,
            tc.tile_pool(name="hpool", bufs=2) as hpool,
            tc.tile_pool(name="spool", bufs=2) as spool,
            tc.tile_pool(name="opool", bufs=4) as opool,
            tc.tile_pool(name="psab", bufs=3, space="PSUM") as psab,
            tc.tile_pool(name="pso", bufs=2, space="PSUM") as pso,
        ):
            # HAM warmup: the PE clock sits at 1.2 GHz until ~3.4us of
            # sustained matmul activity. Burn dummy matmuls on a memset
            # tile while the first weight/x DMAs are in flight so the real
            # matmuls start at 2.4 GHz.
            wu = spool.tile([P, TBLK], BF16, tag="warm", bufs=1)
            nc.vector.memset(wu[:], 0.0)
            pwu = pso.tile([P, TBLK], F32, tag="po", name="pwu")
            for _ in range(40):
                nc.tensor.matmul(pwu[:], wu[:, 0:P], wu[:],
                                 start=True, stop=True)

            # Resident weights. DMA issue order: (w1_k, x0_k) pairs so the
            # first m1 accumulation group can start after ~one pair lands,
            # then w3, w2.
            w1_t = []
            x0_t = []
            for k in range(NDK):
                t1 = wpool.tile([P, H], BF16, tag=f"w1_{k}")
                nc.sync.dma_start(t1[:], w1_d[k * P:(k + 1) * P, :])
                w1_t.append(t1)
                xt = xpool.tile([P, TBLK], BF16, tag=f"x_{k}")
                nc.sync.dma_start(xt[:], xt_d[k * P:(k + 1) * P, 0:TBLK])
                x0_t.append(xt)

            def load_w3():
                w3_t = []
                for k in range(NDK):
                    t3 = wpool.tile([P, H], BF16, tag=f"w3_{k}")
                    nc.sync.dma_start(t3[:], w3_d[k * P:(k + 1) * P, :])
                    w3_t.append(t3)
                return w3_t

            def load_w2():
                w2_t = []
                for k in range(NHM):
                    t2 = wpool.tile([P, D], BF16, tag=f"w2_{k}")
                    nc.sync.dma_start(t2[:], w2_d[k * P:(k + 1) * P, :])
                    w2_t.append(t2)
                return w2_t

            w3_t = w2_t = None

            def load_x(i):
                ts = i * TBLK
                xt_t = []
                for k in range(NDK):
                    xt = xpool.tile([P, TBLK], BF16, tag=f"x_{k}")
                    nc.sync.dma_start(
                        xt[:], xt_d[k * P:(k + 1) * P, ts:ts + TBLK]
                    )
                    xt_t.append(xt)
                return xt_t

            def emit_m3(i, hts):
                """h(i) @ w2 -> out rows [i*TBLK, (i+1)*TBLK)."""
                ts = i * TBLK
                for tm in range(NTSUB):
                    tsub = ts + tm * P
                    for dn in range(NDN):
                        dsl = dn * DBLK
                        po = pso.tile([P, DBLK], F32, tag="po")
                        for hk in range(NHM):
                            nc.tensor.matmul(
                                po[:],
                                hts[hk][:, tm * P:(tm + 1) * P],
                                w2_t[hk][:, dsl:dsl + DBLK],
                                start=(hk == 0), stop=(hk == NHM - 1),
                            )
                        ot = opool.tile([P, DBLK], BF16, tag="o")
                        nc.vector.tensor_copy(ot[:], po[:])
                        nc.sync.dma_start(
                            out_d[tsub:tsub + P, dsl:dsl + DBLK], ot[:]
                        )

            prev_hts = None
            x_next = None
            for i in range(NTBLK):
                if i == 0:
                    xt_t = x0_t
                    # DMA queue order: w3 (needed at ~35us for block 0
                    # pass 2) -> x(1) (needed at ~62us) -> w2 (needed at
                    # ~118us for m3(0)).
                    w3_t = load_w3()
                    x_next = load_x(1)
                    w2_t = load_w2()
                elif i == 1:
                    xt_t = x_next
                else:
                    xt_t = load_x(i)

                # Pass 1: all m1 groups (need only w1 + x), silu on ACT.
                sils = []
                for hm in range(NHM):
                    hs = hm * P
                    pa = psab.tile([P, TBLK], F32, tag="pa")
                    for k in range(NDK):
                        nc.tensor.matmul(
                            pa[:], w1_t[k][:, hs:hs + P], xt_t[k][:],
                            start=(k == 0), stop=(k == NDK - 1),
                        )
                    sil = spool.tile([P, TBLK], BF16, tag=f"sil_{hm}")
                    nc.scalar.activation(
                        sil[:], pa[:], mybir.ActivationFunctionType.Silu
                    )
                    sils.append(sil)
                # Pass 2: all m2 groups (need w3), DVE mul -> h.
                hts = []
                for hm in range(NHM):
                    hs = hm * P
                    pb = psab.tile([P, TBLK], F32, tag="pb")
                    for k in range(NDK):
                        nc.tensor.matmul(
                            pb[:], w3_t[k][:, hs:hs + P], xt_t[k][:],
                            start=(k == 0), stop=(k == NDK - 1),
                        )
                    ht = hpool.tile([P, TBLK], BF16, tag=f"h_{hm}")
                    nc.vector.tensor_mul(ht[:], sils[hm][:], pb[:])
                    hts.append(ht)

                if prev_hts is not None:
                    emit_m3(i - 1, prev_hts)
                prev_hts = hts

            emit_m3(NTBLK - 1, prev_hts)

    nc.compile()
    _CACHE["nc"] = nc
    return nc


def _stage_inputs(x, w1, w2, w3):
    """Per-expert bf16 staging; x pre-transposed to [D, T]."""
    bf = ml_dtypes.bfloat16
    in_maps = []
    for e in range(E):
        in_maps.append({
            "xt": np.ascontiguousarray(x[e].astype(bf).T),
            "w1": np.ascontiguousarray(w1[e].astype(bf)),
            "w3": np.ascontiguousarray(w3[e].astype(bf)),
            "w2": np.ascontiguousarray(w2[e].astype(bf)),
        })
    return in_maps


def kernel(x, w1, w2, w3):
    assert x.shape == (E, T, D) and w1.shape == (E, D, H)
    assert w2.shape == (E, H, D) and w3.shape == (E, D, H)
    nc = _build_module()
    in_maps = _stage_inputs(x, w1, w2, w3)
    res = run_bass_kernel_spmd(nc, in_maps, core_ids=list(range(NCORES)))
    out = np.stack([res.results[e]["out"] for e in range(E)], axis=0)
    return out.astype(np.float32)


# revision 17
# speedup vs baseline: 7.8894x; 1.0036x over previous
"""GroupedExperts (MoE bmm path) forward on 8 Trainium2 NeuronCores.

Reference (per expert e):
    h   = silu(x[e] @ w1[e]) * (x[e] @ w3[e])
    out = h @ w2[e]
with E=8, T=4096, D=2048, H=1024, fp32 inputs.

Sharding: expert-parallel - core e owns expert e (no cross-core traffic).

Schedule (measured ~689 us/core on HW, vs 655 us bf16 PE roofline):
  - out written bf16 (host upcasts): halves output HBM+staging bytes;
    adds ~2e-4 rel err, well within budget.
  - software-pipelined m3: block i's h@w2 matmuls are issued after block
    i+1's m1/m2 matmuls, so the PE never waits on ACT/DVE producing h.
  - w1/w3 staged hm-major on host: each pass-1/2 group gates on one
    contiguous 0.5MB column-block DMA instead of the whole 4MB weight;
    block 0 runs all m1 groups before any m2 group so w3/w2 loads hide
    behind m1 compute. DMA priority: w1[0], x0, w1[1:], w3, x(1), w2.
  - pa/pb PSUM pools triple-buffered (3+3+2 = 8 banks) to keep PSUM
    group-boundary WAR waits off the PE queue.
  - ~40 dummy warmup matmuls during the initial DMA wait release the
    HAM clock gate (1.2 -> 2.4 GHz) before real work arrives.
"""

import numpy as np
import ml_dtypes

import concourse.bass as bass
import concourse.mybir as mybir
import concourse.tile as tile
from concourse import bacc
from concourse.bass_utils import run_bass_kernel_spmd

E, T, D, H = 8, 4096, 2048, 1024
NCORES = 8
P = 128               # partition dim
TBLK = 512            # token block = moving free dim for m1/m2
NTBLK = T // TBLK     # 8
NDK = D // P          # 16 contraction tiles over D
NHM = H // P          # 8 tiles over H
DBLK = 512            # D chunk = moving free dim for m3
NDN = D // DBLK       # 4
NTSUB = TBLK // P     # 4

BF16 = mybir.dt.bfloat16
F32 = mybir.dt.float32

_CACHE: dict = {}


def _build_module():
    if "nc" in _CACHE:
        return _CACHE["nc"]

    nc = bacc.Bacc(
        "TRN2",
        target_bir_lowering=False,
        debug=False,
        enable_asserts=False,
        num_devices=NCORES,
    )

    xt_d = nc.dram_tensor("xt", [D, T], BF16, kind="ExternalInput").ap()
    # w1/w3 staged hm-major: row block hm holds [128, NDK*128] where
    # column block k is w1[k*128:(k+1)*128, hm*128:(hm+1)*128]. One DMA
    # per hm column block -> pass 1/2 group hm gates on 0.5MB, not 4MB.
    w1_d = nc.dram_tensor("w1", [H, D], BF16, kind="ExternalInput").ap()
    w3_d = nc.dram_tensor("w3", [H, D], BF16, kind="ExternalInput").ap()
    w2_d = nc.dram_tensor("w2", [H, D], BF16, kind="ExternalInput").ap()
    out_d = nc.dram_tensor("out", [T, D], BF16, kind="ExternalOutput").ap()

    with tile.TileContext(nc) as tc:
        with (
            tc.tile_pool(name="wpool", bufs=1) as wpool,
            tc.tile_pool(name="xpool", bufs=2) as xpool,
            tc.tile_pool(name="hpool", bufs=2) as hpool,
            tc.tile_pool(name="spool", bufs=2) as spool,
            tc.tile_pool(name="opool", bufs=4) as opool,
            tc.tile_pool(name="psab", bufs=3, space="PSUM") as psab,
            tc.tile_pool(name="pso", bufs=2, space="PSUM") as pso,
        ):
            # HAM warmup: the PE clock sits at 1.2 GHz until ~3.4us of
            # sustained matmul activity. Burn dummy matmuls on a memset
            # tile while the first weight/x DMAs are in flight so the real
            # matmuls start at 2.4 GHz.
            wu = spool.tile([P, TBLK], BF16, tag="warm", bufs=1)
            nc.vector.memset(wu[:], 0.0)
            pwu = pso.tile([P, TBLK], F32, tag="po", name="pwu")
            for _ in range(40):
                nc.tensor.matmul(pwu[:], wu[:, 0:P], wu[:],
                                 start=True, stop=True)

            # Resident weights. DMA issue order: w1[hm=0] then the x0
            # tiles (the hm=0 group gate), then the rest of w1, then w3.
            w1_t = []
            t1 = wpool.tile([P, D], BF16, tag="w1_0", name="t1")
            nc.sync.dma_start(t1[:], w1_d[0:P, :])
            w1_t.append(t1)
            x0_t = []
            for k in range(NDK):
                xt = xpool.tile([P, TBLK], BF16, tag=f"x_{k}")
                nc.sync.dma_start(xt[:], xt_d[k * P:(k + 1) * P, 0:TBLK])
                x0_t.append(xt)
            for hm in range(1, NHM):
                t1 = wpool.tile([P, D], BF16, tag=f"w1_{hm}", name="t1")
                nc.sync.dma_start(t1[:], w1_d[hm * P:(hm + 1) * P, :])
                w1_t.append(t1)

            def load_w3():
                w3_t = []
                for hm in range(NHM):
                    t3 = wpool.tile([P, D], BF16, tag=f"w3_{hm}")
                    nc.sync.dma_start(t3[:], w3_d[hm * P:(hm + 1) * P, :])
                    w3_t.append(t3)
                return w3_t

            def load_w2():
                w2_t = []
                for k in range(NHM):
                    t2 = wpool.tile([P, D], BF16, tag=f"w2_{k}")
                    nc.sync.dma_start(t2[:], w2_d[k * P:(k + 1) * P, :])
                    w2_t.append(t2)
                return w2_t

            w3_t = w2_t = None

            def load_x(i):
                ts = i * TBLK
                xt_t = []
                for k in range(NDK):
                    xt = xpool.tile([P, TBLK], BF16, tag=f"x_{k}")
                    nc.sync.dma_start(
                        xt[:], xt_d[k * P:(k + 1) * P, ts:ts + TBLK]
                    )
                    xt_t.append(xt)
                return xt_t

            def emit_m3(i, hts):
                """h(i) @ w2 -> out rows [i*TBLK, (i+1)*TBLK)."""
                ts = i * TBLK
                for tm in range(NTSUB):
                    tsub = ts + tm * P
                    for dn in range(NDN):
                        dsl = dn * DBLK
                        po = pso.tile([P, DBLK], F32, tag="po")
                        for hk in range(NHM):
                            nc.tensor.matmul(
                                po[:],
                                hts[hk][:, tm * P:(tm + 1) * P],
                                w2_t[hk][:, dsl:dsl + DBLK],
                                start=(hk == 0), stop=(hk == NHM - 1),
                            )
                        ot = opool.tile([P, DBLK], BF16, tag="o")
                        nc.vector.tensor_copy(ot[:], po[:])
                        nc.sync.dma_start(
                            out_d[tsub:tsub + P, dsl:dsl + DBLK], ot[:]
                        )

            prev_hts = None
            x_next = None
            for i in range(NTBLK):
                if i == 0:
                    xt_t = x0_t
                    # DMA queue order: w3 (needed at ~35us for block 0
                    # pass 2) -> x(1) (needed at ~62us) -> w2 (needed at
                    # ~118us for m3(0)).
                    w3_t = load_w3()
                    x_next = load_x(1)
                    w2_t = load_w2()
                elif i == 1:
                    xt_t = x_next
                else:
                    xt_t = load_x(i)

                # Pass 1: all m1 groups (need only w1 + x), silu on ACT.
                sils = []
                for hm in range(NHM):
                    hs = hm * P
                    pa = psab.tile([P, TBLK], F32, tag="pa")
                    for k in range(NDK):
                        nc.tensor.matmul(
                            pa[:], w1_t[hm][:, k * P:(k + 1) * P], xt_t[k][:],
                            start=(k == 0), stop=(k == NDK - 1),
                        )
                    sil = spool.tile([P, TBLK], BF16, tag=f"sil_{hm}")
                    nc.scalar.activation(
                        sil[:], pa[:], mybir.ActivationFunctionType.Silu
                    )
                    sils.append(sil)
                # Pass 2: all m2 groups (need w3), DVE mul -> h.
                hts = []
                for hm in range(NHM):
                    hs = hm * P
                    pb = psab.tile([P, TBLK], F32, tag="pb")
                    for k in range(NDK):
                        nc.tensor.matmul(
                            pb[:], w3_t[hm][:, k * P:(k + 1) * P], xt_t[k][:],
                            start=(k == 0), stop=(k == NDK - 1),
                        )
                    ht = hpool.tile([P, TBLK], BF16, tag=f"h_{hm}")
                    nc.vector.tensor_mul(ht[:], sils[hm][:], pb[:])
                    hts.append(ht)

                if prev_hts is not None:
                    emit_m3(i - 1, prev_hts)
                prev_hts = hts

            emit_m3(NTBLK - 1, prev_hts)

    nc.compile()
    _CACHE["nc"] = nc
    return nc


def _hm_major(w):
    """[D, H] -> [H, D] blocked: row block hm, column block k holds
    w[k*128:(k+1)*128, hm*128:(hm+1)*128]."""
    ndk, nhm = D // P, H // P
    return np.ascontiguousarray(
        w.reshape(ndk, P, nhm, P).transpose(2, 1, 0, 3).reshape(H, D)
    )


def _stage_inputs(x, w1, w2, w3):
    """Per-expert bf16 staging; x pre-transposed to [D, T]."""
    bf = ml_dtypes.bfloat16
    in_maps = []
    for e in range(E):
        in_maps.append({
            "xt": np.ascontiguousarray(x[e].astype(bf).T),
            "w1": _hm_major(w1[e].astype(bf)),
            "w3": _hm_major(w3[e].astype(bf)),
            "w2": np.ascontiguousarray(w2[e].astype(bf)),
        })
    return in_maps


def kernel(x, w1, w2, w3):
    assert x.shape == (E, T, D) and w1.shape == (E, D, H)
    assert w2.shape == (E, H, D) and w3.shape == (E, D, H)
    nc = _build_module()
    in_maps = _stage_inputs(x, w1, w2, w3)
    res = run_bass_kernel_spmd(nc, in_maps, core_ids=list(range(NCORES)))
    out = np.stack([res.results[e]["out"] for e in range(E)], axis=0)
    return out.astype(np.float32)


# revision 19
# speedup vs baseline: 7.9375x; 1.0061x over previous
"""GroupedExperts (MoE bmm path) forward on 8 Trainium2 NeuronCores.

Reference (per expert e):
    h   = silu(x[e] @ w1[e]) * (x[e] @ w3[e])
    out = h @ w2[e]
with E=8, T=4096, D=2048, H=1024, fp32 inputs.

Sharding: expert-parallel - core e owns expert e (no cross-core traffic).

Schedule (measured ~689 us/core on HW, vs 655 us bf16 PE roofline):
  - out written bf16 (host upcasts): halves output HBM+staging bytes;
    adds ~2e-4 rel err, well within budget.
  - software-pipelined m3: block i's h@w2 matmuls are issued after block
    i+1's m1/m2 matmuls, so the PE never waits on ACT/DVE producing h.
  - w1/w3 staged hm-major on host: each pass-1/2 group gates on one
    contiguous 0.5MB column-block DMA instead of the whole 4MB weight;
    block 0 runs all m1 groups before any m2 group so w3/w2 loads hide
    behind m1 compute. DMA priority: w1[0], x0, w1[1:], w3, x(1), w2.
  - pa/pb PSUM pools triple-buffered (3+3+2 = 8 banks) to keep PSUM
    group-boundary WAR waits off the PE queue.
  - ~40 dummy warmup matmuls during the initial DMA wait release the
    HAM clock gate (1.2 -> 2.4 GHz) before real work arrives.
"""

import numpy as np
import ml_dtypes

import concourse.bass as bass
import concourse.mybir as mybir
import concourse.tile as tile
from concourse import bacc
from concourse.bass_utils import run_bass_kernel_spmd

E, T, D, H = 8, 4096, 2048, 1024
NCORES = 8
P = 128               # partition dim
TBLK = 512            # token block = moving free dim for m1/m2
NTBLK = T // TBLK     # 8
NDK = D // P          # 16 contraction tiles over D
NHM = H // P          # 8 tiles over H
DBLK = 512            # D chunk = moving free dim for m3
NDN = D // DBLK       # 4
NTSUB = TBLK // P     # 4

BF16 = mybir.dt.bfloat16
F32 = mybir.dt.float32

_CACHE: dict = {}


def _build_module():
    if "nc" in _CACHE:
        return _CACHE["nc"]

    nc = bacc.Bacc(
        "TRN2",
        target_bir_lowering=False,
        debug=False,
        enable_asserts=False,
        num_devices=NCORES,
    )

    xt_d = nc.dram_tensor("xt", [D, T], BF16, kind="ExternalInput").ap()
    # w1/w3 staged hm-major: row block hm holds [128, NDK*128] where
    # column block k is w1[k*128:(k+1)*128, hm*128:(hm+1)*128]. One DMA
    # per hm column block -> pass 1/2 group hm gates on 0.5MB, not 4MB.
    w1_d = nc.dram_tensor("w1", [H, D], BF16, kind="ExternalInput").ap()
    w3_d = nc.dram_tensor("w3", [H, D], BF16, kind="ExternalInput").ap()
    w2_d = nc.dram_tensor("w2", [H, D], BF16, kind="ExternalInput").ap()
    out_d = nc.dram_tensor("out", [T, D], BF16, kind="ExternalOutput").ap()

    with tile.TileContext(nc) as tc:
        with (
            tc.tile_pool(name="wpool", bufs=1) as wpool,
            tc.tile_pool(name="xpool", bufs=2) as xpool,
            tc.tile_pool(name="hpool", bufs=2) as hpool,
            tc.tile_pool(name="spool", bufs=2) as spool,
            tc.tile_pool(name="opool", bufs=4) as opool,
            tc.tile_pool(name="psab", bufs=3, space="PSUM") as psab,
            tc.tile_pool(name="pso", bufs=2, space="PSUM") as pso,
        ):
            # HAM warmup: the PE clock sits at 1.2 GHz until ~3.4us of
            # sustained matmul activity. Burn dummy matmuls on a memset
            # tile while the first weight/x DMAs are in flight so the real
            # matmuls start at 2.4 GHz.
            wu = spool.tile([P, TBLK], BF16, tag="warm", bufs=1)
            nc.vector.memset(wu[:], 0.0)
            pwu = pso.tile([P, TBLK], F32, tag="po", name="pwu")
            for _ in range(40):
                nc.tensor.matmul(pwu[:], wu[:, 0:P], wu[:],
                                 start=True, stop=True)

            # Resident weights. DMA issue order: w1[hm=0] then the x0
            # tiles (the hm=0 group gate), then the rest of w1, then w3.
            w1_t = []
            t1 = wpool.tile([P, D], BF16, tag="w1_0", name="t1")
            nc.sync.dma_start(t1[:], w1_d[0:P, :])
            w1_t.append(t1)
            x0_t = []
            for k in range(NDK):
                xt = xpool.tile([P, TBLK], BF16, tag=f"x_{k}")
                nc.sync.dma_start(xt[:], xt_d[k * P:(k + 1) * P, 0:TBLK])
                x0_t.append(xt)
            for hm in range(1, NHM):
                t1 = wpool.tile([P, D], BF16, tag=f"w1_{hm}", name="t1")
                nc.sync.dma_start(t1[:], w1_d[hm * P:(hm + 1) * P, :])
                w1_t.append(t1)

            def load_w3():
                w3_t = []
                for hm in range(NHM):
                    t3 = wpool.tile([P, D], BF16, tag=f"w3_{hm}")
                    nc.sync.dma_start(t3[:], w3_d[hm * P:(hm + 1) * P, :])
                    w3_t.append(t3)
                return w3_t

            def load_w2():
                w2_t = []
                for k in range(NHM):
                    t2 = wpool.tile([P, D], BF16, tag=f"w2_{k}")
                    nc.sync.dma_start(t2[:], w2_d[k * P:(k + 1) * P, :])
                    w2_t.append(t2)
                return w2_t

            w3_t = w2_t = None

            def load_x(i):
                ts = i * TBLK
                xt_t = []
                for k in range(NDK):
                    xt = xpool.tile([P, TBLK], BF16, tag=f"x_{k}")
                    nc.sync.dma_start(
                        xt[:], xt_d[k * P:(k + 1) * P, ts:ts + TBLK]
                    )
                    xt_t.append(xt)
                return xt_t

            def emit_m3(i, hts):
                """h(i) @ w2 -> out rows [i*TBLK, (i+1)*TBLK)."""
                ts = i * TBLK
                for tm in range(NTSUB):
                    tsub = ts + tm * P
                    for dn in range(NDN):
                        dsl = dn * DBLK
                        po = pso.tile([P, DBLK], F32, tag="po")
                        for hk in range(NHM):
                            nc.tensor.matmul(
                                po[:],
                                hts[hk][:, tm * P:(tm + 1) * P],
                                w2_t[hk][:, dsl:dsl + DBLK],
                                start=(hk == 0), stop=(hk == NHM - 1),
                            )
                        ot = opool.tile([P, DBLK], BF16, tag="o")
                        nc.vector.tensor_copy(ot[:], po[:])
                        nc.sync.dma_start(
                            out_d[tsub:tsub + P, dsl:dsl + DBLK], ot[:]
                        )

            prev_hts = None
            x_next = None
            for i in range(NTBLK):
                if i == 0:
                    xt_t = x0_t
                    # DMA queue order: w3 (needed at ~35us for block 0
                    # pass 2) -> x(1) (needed at ~62us) -> w2 (needed at
                    # ~118us for m3(0)).
                    w3_t = load_w3()
                    x_next = load_x(1)
                    w2_t = load_w2()
                elif i == 1:
                    xt_t = x_next
                else:
                    xt_t = load_x(i)

                # Pass 1: all m1 groups (need only w1 + x), silu on ACT.
                sils = []
                for hm in range(NHM):
                    hs = hm * P
                    pa = psab.tile([P, TBLK], F32, tag="pa")
                    for k in range(NDK):
                        nc.tensor.matmul(
                            pa[:], w1_t[hm][:, k * P:(k + 1) * P], xt_t[k][:],
                            start=(k == 0), stop=(k == NDK - 1),
                        )
                    sil = spool.tile([P, TBLK], BF16, tag=f"sil_{hm}")
                    nc.scalar.activation(
                        sil[:], pa[:], mybir.ActivationFunctionType.Silu
                    )
                    sils.append(sil)
                # Pass 2: all m2 groups (need w3), DVE mul -> h.
                hts = []
                for hm in range(NHM):
                    hs = hm * P
                    pb = psab.tile([P, TBLK], F32, tag="pb")
                    for k in range(NDK):
                        nc.tensor.matmul(
                            pb[:], w3_t[hm][:, k * P:(k + 1) * P], xt_t[k][:],
                            start=(k == 0), stop=(k == NDK - 1),
                        )
                    ht = hpool.tile([P, TBLK], BF16, tag=f"h_{hm}")
                    nc.vector.tensor_mul(ht[:], sils[hm][:], pb[:])
                    hts.append(ht)

                if prev_hts is not None:
                    emit_m3(i - 1, prev_hts)
                prev_hts = hts

            emit_m3(NTBLK - 1, prev_hts)

    nc.compile()
    _CACHE["nc"] = nc
    return nc


def _hm_major(w):
    """[D, H] -> [H, D] blocked: row block hm, column block k holds
    w[k*128:(k+1)*128, hm*128:(hm+1)*128]."""
    ndk, nhm = D // P, H // P
    return np.ascontiguousarray(
        w.reshape(ndk, P, nhm, P).transpose(2, 1, 0, 3).reshape(H, D)
    )


def _stage_inputs(x, w1, w2, w3):
    """Per-expert bf16 staging; x pre-transposed to [D, T]."""
    bf = ml_dtypes.bfloat16
    in_maps = []
    for e in range(E):
        in_maps.append({
            "xt": np.ascontiguousarray(x[e].astype(bf).T),
            "w1": _hm_major(w1[e].astype(bf)),
            "w3": _hm_major(w3[e].astype(bf)),
            "w2": np.ascontiguousarray(w2[e].astype(bf)),
        })
    return in_maps


def kernel(x, w1, w2, w3):
    assert x.shape == (E, T, D) and w1.shape == (E, D, H)
    assert w2.shape == (E, H, D) and w3.shape == (E, D, H)
    nc = _build_module()
    in_maps = _stage_inputs(x, w1, w2, w3)
    res = run_bass_kernel_spmd(nc, in_maps, core_ids=list(range(NCORES)))
    out = np.stack([res.results[e]["out"] for e in range(E)], axis=0)
    return out.astype(np.float32)
